# revision 9
# baseline (speedup 1.0000x reference)
"""GPT2 attention (B=2,S=2048,D=1024,H=16,hd=64, no causal mask) on 8 trn2 cores.

Sharding: core c handles batch b=c//4 and head-group g=c%4 (4 heads = 2 pairs).
w_attn columns split per head group (Q scaled by 1/sqrt(hd) on host); w_proj
rows split per head group; host sums the 4 partial c_proj outputs per batch.

v2 dataflow (all bf16 on SBUF, fp32 PSUM accumulation):
  host passes hidT [D,S] (pre-transposed) so no on-device hid transpose.
  qT/kT [128,S] per head-pair = wqk_pair.T @ hidT   (one MM per dt chunk)
  V computed directly seq-major: V[k,f] = hidT_chunk.T @ wv  -> vaug
  scores: per (pair,qc512,kt): two row-tiled concurrent MMs (K=64 each,
    heads at array rows 0-63 / 64-127) -> sp[128,1024] -> one ACT exp
    -> eb bf16 -> attnV MMs accumulate op[65,512] (row 64 = denominator).
  normalize: DVE reciprocal_approx_fast on denom row, ones-matmul broadcast,
    DVE mult -> obar (feature-major, pair-packed 128 rows).
  c_proj per 128-query tile: K=128 chains over 2 pairs, interleaved with
    stage B to keep the PE warm; bf16 partial outputs, host sums in f32.
"""

import sys

import numpy as np

if "/opt/trn_rl_repo" not in sys.path:
    sys.path.insert(0, "/opt/trn_rl_repo")

S = 2048
D = 1024
P = 128
NH = 4  # heads per core
HD = 64
N_CORES = 8

_CACHE = {}


def _build_program():
    import concourse.mybir as mybir
    from concourse import bacc
    from concourse.tile import TileContext

    bf16 = mybir.dt.bfloat16
    f32 = mybir.dt.float32
    AF = mybir.ActivationFunctionType
    ALU = mybir.AluOpType

    nc = bacc.Bacc(None, target_bir_lowering=False, debug=False)
    hidT = nc.declare_dram_parameter("hidT", [D, S], bf16, isOutput=False)
    wqkv = nc.declare_dram_parameter("wqkv", [D, 3 * NH * HD], bf16, isOutput=False)
    wp = nc.declare_dram_parameter("wp", [NH * HD, D], bf16, isOutput=False)
    out = nc.declare_dram_parameter("out", [S, D], bf16, isOutput=True)

    with TileContext(nc) as tc:
        with tc.tile_pool(name="const", bufs=1) as constp:
            vaug = constp.tile([P, NH * 16 * 65], bf16)
            # ones column (col 64 of each 65-block) for the softmax denom
            vaug_on = vaug[:, :].rearrange("p (n c) -> p n c", c=65)[:, :, 64:65]
            nc.gpsimd.memset(vaug_on, 1.0)

            hid_sb = [constp.tile([P, S], bf16, name=f"hid{i}") for i in range(8)]
            w_sb = [constp.tile([P, 768], bf16, name=f"w{i}") for i in range(8)]
            wp_sb = [constp.tile([P, D], bf16, name=f"wp{i}") for i in range(2)]
            qT = [constp.tile([P, S], bf16, name=f"qT{i}") for i in range(2)]
            kT = [constp.tile([P, S], bf16, name=f"kT{i}") for i in range(2)]
            obar = [constp.tile([P, S], bf16, name=f"ob{i}") for i in range(2)]

            for i in range(8):
                nc.sync.dma_start(out=hid_sb[i][:], in_=hidT[i * P : (i + 1) * P, :])
                nc.sync.dma_start(out=w_sb[i][:], in_=wqkv[i * P : (i + 1) * P, :])
            for p in range(2):
                nc.sync.dma_start(
                    out=wp_sb[p][:], in_=wp[p * P : (p + 1) * P, :]
                )

            # ---------------- Stage A: qT/kT per pair + V seq-major ------
            with tc.tile_pool(name="qkpsum", bufs=4, space="PSUM") as qkp, \
                 tc.tile_pool(name="vpsum", bufs=3, space="PSUM") as vp:
                for p in range(2):
                    for col, dst in ((p * P, qT[p]), (256 + p * P, kT[p])):
                        for q4 in range(4):
                            ps = qkp.tile([P, 512], f32, tag="qk")
                            for dt_ in range(8):
                                nc.tensor.matmul(
                                    ps[:],
                                    lhsT=w_sb[dt_][:, col : col + P],
                                    rhs=hid_sb[dt_][:, q4 * 512 : (q4 + 1) * 512],
                                    start=(dt_ == 0),
                                    stop=(dt_ == 7),
                                )
                            with nc.allow_low_precision(reason="bf16 qkT"):
                                nc.vector.tensor_copy(
                                    dst[:, q4 * 512 : (q4 + 1) * 512], ps[:]
                                )
                vaug4 = vaug[:, :].rearrange("p (h x) -> p h x", h=NH)
                for kt in range(16):
                    vps = vp.tile([P, NH * HD], f32, tag="v")
                    for dt_ in range(8):
                        nc.tensor.matmul(
                            vps[:],
                            lhsT=hid_sb[dt_][:, kt * P : (kt + 1) * P],
                            rhs=w_sb[dt_][:, 512:768],
                            start=(dt_ == 0),
                            stop=(dt_ == 7),
                        )
                    src = vps[:, :].rearrange("p (h c) -> p h c", h=NH)
                    dst = vaug4[:, :, kt * 65 : kt * 65 + HD]
                    with nc.allow_low_precision(reason="bf16 V"):
                        nc.vector.tensor_copy(dst, src)

            # ---------------- Stages B + C (interleaved) ----------------
            with tc.tile_pool(name="spsum", bufs=2, space="PSUM") as spsum, \
                 tc.tile_pool(name="opsum", bufs=2, space="PSUM") as opsum, \
                 tc.tile_pool(name="ppsum", bufs=2, space="PSUM") as ppp, \
                 tc.tile_pool(name="ebpool", bufs=3) as ebp, \
                 tc.tile_pool(name="recpool", bufs=2) as recp, \
                 tc.tile_pool(name="rbsb", bufs=2) as rbsbp, \
                 tc.tile_pool(name="otpool", bufs=2) as otp:

                def stage_c(qc):
                    for j in range(4):
                        qt = qc * 4 + j
                        ot = otp.tile([P, D], bf16, tag="ot")
                        for ec in range(2):
                            pp = ppp.tile([P, 512], f32, tag="pp")
                            for p in range(2):
                                nc.tensor.matmul(
                                    pp[:],
                                    lhsT=obar[p][:, qt * P : (qt + 1) * P],
                                    rhs=wp_sb[p][:, ec * 512 : (ec + 1) * 512],
                                    start=(p == 0),
                                    stop=(p == 1),
                                )
                            with nc.allow_low_precision(reason="bf16 out"):
                                nc.vector.tensor_copy(
                                    ot[:, ec * 512 : (ec + 1) * 512], pp[:]
                                )
                        nc.sync.dma_start(
                            out=out[qt * P : (qt + 1) * P, :], in_=ot[:]
                        )

                for qc in range(4):
                    q0 = qc * 512
                    for p in range(2):
                        ops = [
                            opsum.tile([65, 512], f32, tag="op", name=f"op{u}")
                            for u in range(2)
                        ]
                        for kt in range(16):
                            sp = spsum.tile([P, 1024], f32, tag="sp")
                            for u in range(2):
                                r0 = u * HD
                                nc.tensor.matmul(
                                    sp[:, u * 512 : (u + 1) * 512],
                                    lhsT=kT[p][r0 : r0 + HD, kt * P : (kt + 1) * P],
                                    rhs=qT[p][r0 : r0 + HD, q0 : q0 + 512],
                                    start=True,
                                    stop=True,
                                )
                            eb = ebp.tile([P, 1024], bf16, tag="eb")
                            with nc.allow_low_precision(reason="bf16 exp"):
                                nc.scalar.activation(eb[:], sp[:], AF.Exp)
                            for u in range(2):
                                base = ((2 * p + u) * 16 + kt) * 65
                                nc.tensor.matmul(
                                    ops[u][:],
                                    lhsT=vaug[:, base : base + 65],
                                    rhs=eb[:, u * 512 : (u + 1) * 512],
                                    start=(kt == 0),
                                    stop=(kt == 15),
                                )
                        for u in range(2):
                            # evacuate op to SBUF promptly so the PSUM bank
                            # frees for the next block; the slow reciprocal
                            # then runs entirely in SBUF off the PE path
                            ou = recp.tile([65, 512], f32, tag="ou")
                            nc.vector.tensor_copy(ou[:], ops[u][:])
                            rec = recp.tile([1, 512], f32, tag="rec")
                            with nc.allow_low_precision(
                                reason="softmax denom recip"
                            ):
                                nc.vector.reciprocal(
                                    rec[:], ou[HD : HD + 1, :]
                                )
                            rb_s = rbsbp.tile([HD, 512], f32, tag="rbsb")
                            nc.gpsimd.partition_broadcast(
                                rb_s[:], rec[0:1, :], channels=HD
                            )
                            with nc.allow_low_precision(reason="bf16 obar"):
                                nc.vector.tensor_tensor(
                                    out=obar[p][
                                        u * HD : (u + 1) * HD, q0 : q0 + 512
                                    ],
                                    in0=ou[0:HD, :],
                                    in1=rb_s[:],
                                    op=ALU.mult,
                                )
                    if qc > 0:
                        stage_c(qc - 1)
                stage_c(3)

    nc.compile()
    return nc


def _get_nc():
    if "nc" not in _CACHE:
        _CACHE["nc"] = _build_program()
    return _CACHE["nc"]


def _shard_inputs(hidden_states, w_attn, w_proj):
    import ml_dtypes

    bf16 = ml_dtypes.bfloat16
    scale = 1.0 / np.sqrt(np.float32(HD))
    in_maps = []
    for c in range(N_CORES):
        b, g = divmod(c, 4)
        cs = slice(g * NH * HD, (g + 1) * NH * HD)
        wq = w_attn[:, 0:D][:, cs] * scale
        wk = w_attn[:, D : 2 * D][:, cs]
        wv = w_attn[:, 2 * D : 3 * D][:, cs]
        in_maps.append(
            {
                "hidT": np.ascontiguousarray(
                    hidden_states[b].T.astype(bf16)
                ),
                "wqkv": np.ascontiguousarray(
                    np.concatenate([wq, wk, wv], axis=1).astype(bf16)
                ),
                "wp": np.ascontiguousarray(w_proj[cs, :].astype(bf16)),
            }
        )
    return in_maps


def run(hidden_states, w_attn, w_proj, trace=False):
    from concourse.bass_utils import run_bass_kernel_spmd

    nc = _get_nc()
    in_maps = _shard_inputs(hidden_states, w_attn, w_proj)
    res = run_bass_kernel_spmd(nc, in_maps, list(range(N_CORES)), trace=trace)
    parts = [res.results[c]["out"].astype(np.float32) for c in range(N_CORES)]
    out = np.stack(
        [
            parts[0] + parts[1] + parts[2] + parts[3],
            parts[4] + parts[5] + parts[6] + parts[7],
        ]
    ).astype(np.float32)
    return out, res


def kernel(hidden_states, w_attn, w_proj):
    out, _ = run(
        np.asarray(hidden_states), np.asarray(w_attn), np.asarray(w_proj)
    )
    return out


# revision 13
# speedup vs baseline: 1.0125x; 1.0125x over previous
"""GPT2 attention (B=2,S=2048,D=1024,H=16,hd=64, no causal mask) on 8 trn2 cores.

Sharding: core c handles batch b=c//4 and head-group g=c%4 (4 heads = 2 pairs).
w_attn columns split per head group (Q scaled by 1/sqrt(hd) on host); w_proj
rows split per head group; host sums the 4 partial c_proj outputs per batch.

v2 dataflow (all bf16 on SBUF, fp32 PSUM accumulation):
  host passes hidT [D,S] (pre-transposed) so no on-device hid transpose.
  qT/kT [128,S] per head-pair = wqk_pair.T @ hidT   (one MM per dt chunk)
  V computed directly seq-major: V[k,f] = hidT_chunk.T @ wv  -> vaug
  scores: per (pair,qc512,kt): two row-tiled concurrent MMs (K=64 each,
    heads at array rows 0-63 / 64-127) -> sp[128,1024] -> one ACT exp
    -> eb bf16 -> attnV MMs accumulate op[65,512] (row 64 = denominator).
  normalize: DVE reciprocal_approx_fast on denom row, ones-matmul broadcast,
    DVE mult -> obar (feature-major, pair-packed 128 rows).
  c_proj per 128-query tile: K=128 chains over 2 pairs, interleaved with
    stage B to keep the PE warm; bf16 partial outputs, host sums in f32.
"""

import sys

import numpy as np

if "/opt/trn_rl_repo" not in sys.path:
    sys.path.insert(0, "/opt/trn_rl_repo")

S = 2048
D = 1024
P = 128
NH = 4  # heads per core
HD = 64
N_CORES = 8

_CACHE = {}


def _build_program():
    import concourse.mybir as mybir
    from concourse import bacc
    from concourse.tile import TileContext

    bf16 = mybir.dt.bfloat16
    f32 = mybir.dt.float32
    AF = mybir.ActivationFunctionType
    ALU = mybir.AluOpType

    nc = bacc.Bacc(None, target_bir_lowering=False, debug=False)
    hidT = nc.declare_dram_parameter("hidT", [D, S], bf16, isOutput=False)
    wqkv = nc.declare_dram_parameter("wqkv", [D, 3 * NH * HD], bf16, isOutput=False)
    wp = nc.declare_dram_parameter("wp", [NH * HD, D], bf16, isOutput=False)
    out = nc.declare_dram_parameter("out", [S, D], bf16, isOutput=True)

    with TileContext(nc) as tc:
        with tc.tile_pool(name="const", bufs=1) as constp:
            vaug = constp.tile([P, NH * 16 * 65], bf16)
            # ones column (col 64 of each 65-block) for the softmax denom
            vaug_on = vaug[:, :].rearrange("p (n c) -> p n c", c=65)[:, :, 64:65]
            nc.gpsimd.memset(vaug_on, 1.0)

            hid_sb = [constp.tile([P, S], bf16, name=f"hid{i}") for i in range(8)]
            w_sb = [constp.tile([P, 768], bf16, name=f"w{i}") for i in range(8)]
            wp_sb = [constp.tile([P, D], bf16, name=f"wp{i}") for i in range(2)]
            qT = [constp.tile([P, S], bf16, name=f"qT{i}") for i in range(2)]
            kT = [constp.tile([P, S], bf16, name=f"kT{i}") for i in range(2)]
            obar = [constp.tile([P, S], bf16, name=f"ob{i}") for i in range(2)]

            for i in range(8):
                nc.sync.dma_start(out=hid_sb[i][:], in_=hidT[i * P : (i + 1) * P, :])
                nc.sync.dma_start(out=w_sb[i][:], in_=wqkv[i * P : (i + 1) * P, :])
            for p in range(2):
                nc.sync.dma_start(
                    out=wp_sb[p][:], in_=wp[p * P : (p + 1) * P, :]
                )

            # ---------------- Stage A: qT/kT per pair + V seq-major ------
            with tc.tile_pool(name="qkpsum", bufs=4, space="PSUM") as qkp, \
                 tc.tile_pool(name="vpsum", bufs=3, space="PSUM") as vp:
                def qk_chains(p):
                    for col, dst in ((p * P, qT[p]), (256 + p * P, kT[p])):
                        for q4 in range(4):
                            ps = qkp.tile([P, 512], f32, tag="qk")
                            for dt_ in range(8):
                                nc.tensor.matmul(
                                    ps[:],
                                    lhsT=w_sb[dt_][:, col : col + P],
                                    rhs=hid_sb[dt_][:, q4 * 512 : (q4 + 1) * 512],
                                    start=(dt_ == 0),
                                    stop=(dt_ == 7),
                                )
                            with nc.allow_low_precision(reason="bf16 qkT"):
                                nc.vector.tensor_copy(
                                    dst[:, q4 * 512 : (q4 + 1) * 512], ps[:]
                                )

                # pair0 Q/K first, then V, then pair1 Q/K: lets stage B's
                # first exps start ~15us earlier (ACT idle shrink)
                qk_chains(0)
                vaug4 = vaug[:, :].rearrange("p (h x) -> p h x", h=NH)
                for kt in range(16):
                    vps = vp.tile([P, NH * HD], f32, tag="v")
                    for dt_ in range(8):
                        nc.tensor.matmul(
                            vps[:],
                            lhsT=hid_sb[dt_][:, kt * P : (kt + 1) * P],
                            rhs=w_sb[dt_][:, 512:768],
                            start=(dt_ == 0),
                            stop=(dt_ == 7),
                        )
                    src = vps[:, :].rearrange("p (h c) -> p h c", h=NH)
                    dst = vaug4[:, :, kt * 65 : kt * 65 + HD]
                    with nc.allow_low_precision(reason="bf16 V"):
                        nc.vector.tensor_copy(dst, src)
                qk_chains(1)

            # ---------------- Stages B + C (interleaved) ----------------
            with tc.tile_pool(name="spsum", bufs=2, space="PSUM") as spsum, \
                 tc.tile_pool(name="opsum", bufs=2, space="PSUM") as opsum, \
                 tc.tile_pool(name="ppsum", bufs=2, space="PSUM") as ppp, \
                 tc.tile_pool(name="ebpool", bufs=3) as ebp, \
                 tc.tile_pool(name="recpool", bufs=2) as recp, \
                 tc.tile_pool(name="rbsb", bufs=2) as rbsbp, \
                 tc.tile_pool(name="otpool", bufs=2) as otp:

                def stage_c(qc):
                    for j in range(4):
                        qt = qc * 4 + j
                        ot = otp.tile([P, D], bf16, tag="ot")
                        for ec in range(2):
                            pp = ppp.tile([P, 512], f32, tag="pp")
                            for p in range(2):
                                nc.tensor.matmul(
                                    pp[:],
                                    lhsT=obar[p][:, qt * P : (qt + 1) * P],
                                    rhs=wp_sb[p][:, ec * 512 : (ec + 1) * 512],
                                    start=(p == 0),
                                    stop=(p == 1),
                                )
                            with nc.allow_low_precision(reason="bf16 out"):
                                nc.vector.tensor_copy(
                                    ot[:, ec * 512 : (ec + 1) * 512], pp[:]
                                )
                        nc.sync.dma_start(
                            out=out[qt * P : (qt + 1) * P, :], in_=ot[:]
                        )

                for qc in range(4):
                    q0 = qc * 512
                    for p in range(2):
                        ops = [
                            opsum.tile([65, 512], f32, tag="op", name=f"op{u}")
                            for u in range(2)
                        ]
                        for kt in range(16):
                            sp = spsum.tile([P, 1024], f32, tag="sp")
                            for u in range(2):
                                r0 = u * HD
                                nc.tensor.matmul(
                                    sp[:, u * 512 : (u + 1) * 512],
                                    lhsT=kT[p][r0 : r0 + HD, kt * P : (kt + 1) * P],
                                    rhs=qT[p][r0 : r0 + HD, q0 : q0 + 512],
                                    start=True,
                                    stop=True,
                                )
                            eb = ebp.tile([P, 1024], bf16, tag="eb")
                            with nc.allow_low_precision(reason="bf16 exp"):
                                nc.scalar.activation(eb[:], sp[:], AF.Exp)
                            for u in range(2):
                                base = ((2 * p + u) * 16 + kt) * 65
                                nc.tensor.matmul(
                                    ops[u][:],
                                    lhsT=vaug[:, base : base + 65],
                                    rhs=eb[:, u * 512 : (u + 1) * 512],
                                    start=(kt == 0),
                                    stop=(kt == 15),
                                )
                        for u in range(2):
                            # evacuate op to SBUF promptly so the PSUM bank
                            # frees for the next block; the slow reciprocal
                            # then runs entirely in SBUF off the PE path
                            ou = recp.tile([65, 512], f32, tag="ou")
                            nc.vector.tensor_copy(ou[:], ops[u][:])
                            # 1/d = exp(-ln(d)) on ACT: same table set as Exp,
                            # ~0.6us/op vs 3.3us DVE reciprocal that also
                            # blocked the DVE FIFO at block boundaries
                            ln_t = recp.tile([1, 512], f32, tag="ln")
                            nc.scalar.activation(
                                ln_t[:], ou[HD : HD + 1, :], AF.Ln
                            )
                            rec = recp.tile([1, 512], f32, tag="rec")
                            nc.scalar.activation(
                                rec[:], ln_t[0:1, :], AF.Exp, scale=-1.0
                            )
                            rb_s = rbsbp.tile([HD, 512], f32, tag="rbsb")
                            nc.gpsimd.partition_broadcast(
                                rb_s[:], rec[0:1, :], channels=HD
                            )
                            with nc.allow_low_precision(reason="bf16 obar"):
                                nc.vector.tensor_tensor(
                                    out=obar[p][
                                        u * HD : (u + 1) * HD, q0 : q0 + 512
                                    ],
                                    in0=ou[0:HD, :],
                                    in1=rb_s[:],
                                    op=ALU.mult,
                                )
                        if p == 0 and qc > 0:
                            stage_c(qc - 1)
                stage_c(3)

    nc.compile()
    return nc


def _get_nc():
    if "nc" not in _CACHE:
        _CACHE["nc"] = _build_program()
    return _CACHE["nc"]


def _shard_inputs(hidden_states, w_attn, w_proj):
    import ml_dtypes

    bf16 = ml_dtypes.bfloat16
    scale = 1.0 / np.sqrt(np.float32(HD))
    in_maps = []
    for c in range(N_CORES):
        b, g = divmod(c, 4)
        cs = slice(g * NH * HD, (g + 1) * NH * HD)
        wq = w_attn[:, 0:D][:, cs] * scale
        wk = w_attn[:, D : 2 * D][:, cs]
        wv = w_attn[:, 2 * D : 3 * D][:, cs]
        in_maps.append(
            {
                "hidT": np.ascontiguousarray(
                    hidden_states[b].T.astype(bf16)
                ),
                "wqkv": np.ascontiguousarray(
                    np.concatenate([wq, wk, wv], axis=1).astype(bf16)
                ),
                "wp": np.ascontiguousarray(w_proj[cs, :].astype(bf16)),
            }
        )
    return in_maps


def run(hidden_states, w_attn, w_proj, trace=False):
    from concourse.bass_utils import run_bass_kernel_spmd

    nc = _get_nc()
    in_maps = _shard_inputs(hidden_states, w_attn, w_proj)
    res = run_bass_kernel_spmd(nc, in_maps, list(range(N_CORES)), trace=trace)
    parts = [res.results[c]["out"].astype(np.float32) for c in range(N_CORES)]
    out = np.stack(
        [
            parts[0] + parts[1] + parts[2] + parts[3],
            parts[4] + parts[5] + parts[6] + parts[7],
        ]
    ).astype(np.float32)
    return out, res


def kernel(hidden_states, w_attn, w_proj):
    out, _ = run(
        np.asarray(hidden_states), np.asarray(w_attn), np.asarray(w_proj)
    )
    return out


# revision 15
# speedup vs baseline: 1.0331x; 1.0204x over previous
"""GPT2 attention (B=2,S=2048,D=1024,H=16,hd=64, no causal mask) on 8 trn2 cores.

Sharding: core c handles batch b=c//4 and head-group g=c%4 (4 heads = 2 pairs).
w_attn columns split per head group (Q scaled by 1/sqrt(hd) on host); w_proj
rows split per head group; host sums the 4 partial c_proj outputs per batch.

v2 dataflow (all bf16 on SBUF, fp32 PSUM accumulation):
  host passes hidT [D,S] (pre-transposed) so no on-device hid transpose.
  qT/kT [128,S] per head-pair = wqk_pair.T @ hidT   (one MM per dt chunk)
  V computed directly seq-major: V[k,f] = hidT_chunk.T @ wv  -> vaug
  scores: per (pair,qc512,kt): two row-tiled concurrent MMs (K=64 each,
    heads at array rows 0-63 / 64-127) -> sp[128,1024] -> one ACT exp
    -> eb bf16 -> attnV MMs accumulate op[65,512] (row 64 = denominator).
  normalize: DVE reciprocal_approx_fast on denom row, ones-matmul broadcast,
    DVE mult -> obar (feature-major, pair-packed 128 rows).
  c_proj per 128-query tile: K=128 chains over 2 pairs, interleaved with
    stage B to keep the PE warm; bf16 partial outputs, host sums in f32.
"""

import sys

import numpy as np

if "/opt/trn_rl_repo" not in sys.path:
    sys.path.insert(0, "/opt/trn_rl_repo")

S = 2048
D = 1024
P = 128
NH = 4  # heads per core
HD = 64
N_CORES = 8

_CACHE = {}


def _build_program():
    import concourse.mybir as mybir
    from concourse import bacc
    from concourse.tile import TileContext

    bf16 = mybir.dt.bfloat16
    f32 = mybir.dt.float32
    AF = mybir.ActivationFunctionType
    ALU = mybir.AluOpType

    nc = bacc.Bacc(None, target_bir_lowering=False, debug=False)
    hidT = nc.declare_dram_parameter("hidT", [D, S], bf16, isOutput=False)
    wqkv = nc.declare_dram_parameter("wqkv", [D, 3 * NH * HD], bf16, isOutput=False)
    wp = nc.declare_dram_parameter("wp", [NH * HD, D], bf16, isOutput=False)
    out = nc.declare_dram_parameter("out", [S, D], bf16, isOutput=True)

    with TileContext(nc) as tc:
        with tc.tile_pool(name="const", bufs=1) as constp:
            vaug = constp.tile([P, NH * 16 * 65], bf16)
            # ones column (col 64 of each 65-block) for the softmax denom
            vaug_on = vaug[:, :].rearrange("p (n c) -> p n c", c=65)[:, :, 64:65]
            nc.gpsimd.memset(vaug_on, 1.0)

            hid_sb = [constp.tile([P, S], bf16, name=f"hid{i}") for i in range(8)]
            w_sb = [constp.tile([P, 768], bf16, name=f"w{i}") for i in range(8)]
            wp_sb = [constp.tile([P, D], bf16, name=f"wp{i}") for i in range(2)]
            qT = [constp.tile([P, S], bf16, name=f"qT{i}") for i in range(2)]
            kT = [constp.tile([P, S], bf16, name=f"kT{i}") for i in range(2)]
            obar = [constp.tile([P, S], bf16, name=f"ob{i}") for i in range(2)]

            for i in range(8):
                nc.sync.dma_start(out=hid_sb[i][:], in_=hidT[i * P : (i + 1) * P, :])
                nc.sync.dma_start(out=w_sb[i][:], in_=wqkv[i * P : (i + 1) * P, :])
            for p in range(2):
                nc.sync.dma_start(
                    out=wp_sb[p][:], in_=wp[p * P : (p + 1) * P, :]
                )

            # ---------------- Stage A: qT/kT per pair + V seq-major ------
            with tc.tile_pool(name="qkpsum", bufs=4, space="PSUM") as qkp, \
                 tc.tile_pool(name="vpsum", bufs=3, space="PSUM") as vp:
                def qk_chains(p):
                    for col, dst in ((p * P, qT[p]), (256 + p * P, kT[p])):
                        for q4 in range(4):
                            ps = qkp.tile([P, 512], f32, tag="qk")
                            for dt_ in range(8):
                                nc.tensor.matmul(
                                    ps[:],
                                    lhsT=w_sb[dt_][:, col : col + P],
                                    rhs=hid_sb[dt_][:, q4 * 512 : (q4 + 1) * 512],
                                    start=(dt_ == 0),
                                    stop=(dt_ == 7),
                                )
                            with nc.allow_low_precision(reason="bf16 qkT"):
                                nc.vector.tensor_copy(
                                    dst[:, q4 * 512 : (q4 + 1) * 512], ps[:]
                                )

                # pair0 Q/K first, then V, then pair1 Q/K: lets stage B's
                # first exps start ~15us earlier (ACT idle shrink)
                qk_chains(0)
                vaug4 = vaug[:, :].rearrange("p (h x) -> p h x", h=NH)
                for kt in range(16):
                    vps = vp.tile([P, NH * HD], f32, tag="v")
                    for dt_ in range(8):
                        nc.tensor.matmul(
                            vps[:],
                            lhsT=hid_sb[dt_][:, kt * P : (kt + 1) * P],
                            rhs=w_sb[dt_][:, 512:768],
                            start=(dt_ == 0),
                            stop=(dt_ == 7),
                        )
                    src = vps[:, :].rearrange("p (h c) -> p h c", h=NH)
                    dst = vaug4[:, :, kt * 65 : kt * 65 + HD]
                    with nc.allow_low_precision(reason="bf16 V"):
                        nc.vector.tensor_copy(dst, src)
                qk_chains(1)

            # ---------------- Stages B + C (software pipelined) ----------
            # 8 blocks (qc, pair), each 16 kt steps; adjacent blocks overlap
            # by 8 steps so one block's normalize/exp tail always hides under
            # the next block's body (no all-engine boundary stalls).
            # PSUM: sp 2x[128,1024]=4 banks + oppp 4x[128,512]=4 banks.
            with tc.tile_pool(name="spsum", bufs=2, space="PSUM") as spsum, \
                 tc.tile_pool(name="oppp", bufs=4, space="PSUM") as oppp, \
                 tc.tile_pool(name="ebpool", bufs=4) as ebp, \
                 tc.tile_pool(name="recpool", bufs=2) as recp, \
                 tc.tile_pool(name="rbsb", bufs=2) as rbsbp, \
                 tc.tile_pool(name="otpool", bufs=2) as otp:

                def stage_c(qc):
                    for j in range(4):
                        qt = qc * 4 + j
                        ot = otp.tile([P, D], bf16, tag="ot")
                        for ec in range(2):
                            pp = oppp.tile([P, 512], f32, tag="op", name="pp")
                            for p in range(2):
                                nc.tensor.matmul(
                                    pp[:],
                                    lhsT=obar[p][:, qt * P : (qt + 1) * P],
                                    rhs=wp_sb[p][:, ec * 512 : (ec + 1) * 512],
                                    start=(p == 0),
                                    stop=(p == 1),
                                )
                            with nc.allow_low_precision(reason="bf16 out"):
                                nc.vector.tensor_copy(
                                    ot[:, ec * 512 : (ec + 1) * 512], pp[:]
                                )
                        nc.sync.dma_start(
                            out=out[qt * P : (qt + 1) * P, :], in_=ot[:]
                        )

                blocks = [(qc, p) for qc in range(4) for p in range(2)]
                ops_of = {}

                def emit_kt(bi, kt):
                    qc, p = blocks[bi]
                    q0 = qc * 512
                    if kt == 0:
                        ops_of[bi] = [
                            oppp.tile([65, 512], f32, tag="op", name=f"op{u}")
                            for u in range(2)
                        ]
                    ops = ops_of[bi]
                    sp = spsum.tile([P, 1024], f32, tag="sp", name="sp")
                    for u in range(2):
                        r0 = u * HD
                        nc.tensor.matmul(
                            sp[:, u * 512 : (u + 1) * 512],
                            lhsT=kT[p][r0 : r0 + HD, kt * P : (kt + 1) * P],
                            rhs=qT[p][r0 : r0 + HD, q0 : q0 + 512],
                            start=True,
                            stop=True,
                        )
                    eb = ebp.tile([P, 1024], bf16, tag="eb", name="eb")
                    with nc.allow_low_precision(reason="bf16 exp"):
                        nc.scalar.activation(eb[:], sp[:], AF.Exp)
                    for u in range(2):
                        base = ((2 * p + u) * 16 + kt) * 65
                        nc.tensor.matmul(
                            ops[u][:],
                            lhsT=vaug[:, base : base + 65],
                            rhs=eb[:, u * 512 : (u + 1) * 512],
                            start=(kt == 0),
                            stop=(kt == 15),
                        )

                def emit_norm(bi):
                    qc, p = blocks[bi]
                    q0 = qc * 512
                    for u in range(2):
                        # evacuate op to SBUF fast (frees the PSUM slot);
                        # the slow reciprocal runs in SBUF off the PE path
                        ou = recp.tile([65, 512], f32, tag="ou", name="ou")
                        nc.vector.tensor_copy(ou[:], ops_of[bi][u][:])
                        rec = recp.tile([1, 512], f32, tag="rec", name="rec")
                        with nc.allow_low_precision(reason="denom recip"):
                            nc.vector.reciprocal(rec[:], ou[HD : HD + 1, :])
                        rb_s = rbsbp.tile([HD, 512], f32, tag="rbsb", name="rb")
                        nc.gpsimd.partition_broadcast(
                            rb_s[:], rec[0:1, :], channels=HD
                        )
                        with nc.allow_low_precision(reason="bf16 obar"):
                            nc.vector.tensor_tensor(
                                out=obar[p][u * HD : (u + 1) * HD, q0 : q0 + 512],
                                in0=ou[0:HD, :],
                                in1=rb_s[:],
                                op=ALU.mult,
                            )
                    del ops_of[bi]

                OFF = 8  # block i starts at step 8*i
                for t in range(OFF * 7 + 17):
                    for bi in range(8):
                        kt = t - OFF * bi
                        if 0 <= kt < 16:
                            emit_kt(bi, kt)
                        elif kt == 16:
                            emit_norm(bi)
                            if bi % 2 == 1:
                                stage_c(bi // 2)

    nc.compile()
    return nc


def _get_nc():
    if "nc" not in _CACHE:
        _CACHE["nc"] = _build_program()
    return _CACHE["nc"]


def _shard_inputs(hidden_states, w_attn, w_proj):
    import ml_dtypes

    bf16 = ml_dtypes.bfloat16
    scale = 1.0 / np.sqrt(np.float32(HD))
    in_maps = []
    for c in range(N_CORES):
        b, g = divmod(c, 4)
        cs = slice(g * NH * HD, (g + 1) * NH * HD)
        wq = w_attn[:, 0:D][:, cs] * scale
        wk = w_attn[:, D : 2 * D][:, cs]
        wv = w_attn[:, 2 * D : 3 * D][:, cs]
        in_maps.append(
            {
                "hidT": np.ascontiguousarray(
                    hidden_states[b].T.astype(bf16)
                ),
                "wqkv": np.ascontiguousarray(
                    np.concatenate([wq, wk, wv], axis=1).astype(bf16)
                ),
                "wp": np.ascontiguousarray(w_proj[cs, :].astype(bf16)),
            }
        )
    return in_maps


def run(hidden_states, w_attn, w_proj, trace=False):
    from concourse.bass_utils import run_bass_kernel_spmd

    nc = _get_nc()
    in_maps = _shard_inputs(hidden_states, w_attn, w_proj)
    res = run_bass_kernel_spmd(nc, in_maps, list(range(N_CORES)), trace=trace)
    parts = [res.results[c]["out"].astype(np.float32) for c in range(N_CORES)]
    out = np.stack(
        [
            parts[0] + parts[1] + parts[2] + parts[3],
            parts[4] + parts[5] + parts[6] + parts[7],
        ]
    ).astype(np.float32)
    return out, res


def kernel(hidden_states, w_attn, w_proj):
    out, _ = run(
        np.asarray(hidden_states), np.asarray(w_attn), np.asarray(w_proj)
    )
    return out
